# revision 34
# baseline (speedup 1.0000x reference)
"""Bass/Trainium2 kernel for BoundaryAwareDownConv.

Computation (see reference): for x[B=8, T=8192, D=512] with a space token at
every position t % 8 == 7, pool each 8-token segment by the mean of its 7
non-space tokens -> pooled[B, W=1024, D], then proj = pooled @ w_proj.T +
b_proj, then LayerNorm(D) * gamma + beta.

Sharding: data-parallel over batch, one batch row per NeuronCore (8 cores).
Params are replicated.

v7 pipeline (per core, x row staged as fp16 [8192, 512]):
  - 4 double-chunk DMAs on the SP HWDGE ring, each [128 partitions, 2 seg,
    8 tok, 512] fp16: partition p holds segments p and p+128 of the
    double-chunk, two contiguous 8 KB descriptors per partition (line-rate;
    the space row is loaded but excluded from the add tree).
  - Pooling as a 4-op fp16 add tree on DVE over both chunks at once
    (halves the per-op launch overhead). Scale 1/7 is folded into the
    staged w.
  - The staged projection weights are CENTERED host-side (wT - rowmean,
    b - mean(b)): the matmul then directly produces proj - mean(proj)
    (exact - the LN mean is linear in the weights), so no mean pass is
    needed on device.
  - Per 128-segment chunk: 4 PE fp16 transposes into one PSUM bank, psum->
    SBUF copies split between ACT and DVE, 4 K=128 fp16 matmuls pooledT.T @
    wTc into PSUM f32, bias add on DVE (broadcast-staged b), sum-of-squares
    on ACT (Square + row-accumulate), Sqrt on ACT, reciprocal on DVE,
    scale-only apply on ACT, fp16 out tile.
  - Emission is software-pipelined: chunk R's late stages are emitted ~4
    chunks behind the loads/pooling so late-stage ops never block a later
    chunk's early-stage ops in engine program order.
  - Output DMAs ride the ACT HWDGE ring (no head-of-line blocking of the SP
    ring's x loads); out is staged fp16 and upcast to f32 on the host.
"""

import numpy as np

B, T, D = 8, 8192, 512
STRIDE = 8
W = T // STRIDE  # 1024
LN_EPS = 1e-5
N_CORES = 8
N_CHUNKS = 8         # 128 segments (= 1024 tokens) per chunk
N_DBL = N_CHUNKS // 2
VALID = STRIDE - 1   # 7 non-space tokens per segment
# fp16 pooled tiles / transposes: halves the DVE write cost of the final
# pooling add; flip to False to fall back to the float32r transpose path.
PM_F16 = True


def _patched_tile_context(tile, mybir, ScopedClock):
    """TileContext whose kernel-tail drain carries no sem waits.

    The walrus build in this container rejects sync-wait commands on Drain
    instructions (setupSyncWait<...NO_STRUCT>: "Too many sync wait commands").
    Stock TileContext parks the global-clock catch-up waits on the SP Drain;
    park them on SP nops (one wait each) instead.
    """

    class PatchedTileContext(tile.TileContext):
        def _drain_and_barrier(self, tick_clock, wait_clock):
            required = ScopedClock({None: tick_clock.global_clock})
            carrier = self.nc.sync.nop(nofuse=True)
            wait_clock.add_sem_waits(carrier.ins, required)
            si = carrier.ins.sync_info
            waits = list(si.on_wait) if si is not None else []
            if len(waits) > 1:
                si.on_wait = waits[:1]
                carrier.ins.sync_info = si
                for w in waits[1:]:
                    extra = self.nc.sync.nop(nofuse=True)
                    extra.ins.sync_info = mybir.SyncInfo(on_wait=[w], on_update=[])
            # The carrier nops run earlier on the same (SP) engine, so the
            # drain transitively waits on everything without carrying waits.
            self.nc.sync.drain()
            self.nc.all_engine_barrier()
            assert self.sems is not None
            popped = self.nc._tile_sem_poison_stack.pop()
            assert popped is self._sem_poison
            self.nc.clear_and_free_semaphores(list(self.sems.allocated().values()))
            self.nc.all_engine_barrier()

    return PatchedTileContext


def _split_multi_waits(nc, mybir):
    """Rewrite the scheduled BIR so no instruction carries more than one sync
    wait (and Drain carries none): the walrus build here rejects them
    (setupSyncWait: "Too many sync wait commands"). Surplus waits move onto
    same-engine InstNoOp carriers placed immediately before the instruction -
    same-engine program order preserves the blocking semantics."""
    n = 0
    for fn in nc.m.functions:
        for bb in fn.blocks:
            changed = False
            new_insts = []
            for inst in bb.instructions:
                si = inst.sync_info
                waits = list(si.on_wait) if si is not None else []
                limit = 0 if inst.opcode == "Drain" else 1
                if len(waits) > limit:
                    changed = True
                    for w in waits[limit:]:
                        n += 1
                        new_insts.append(
                            mybir.InstNoOp(
                                name=f"wsplit_{n}_{inst.name}",
                                engine=inst.engine,
                                sync_info=mybir.SyncInfo(on_wait=[w], on_update=[]),
                                bass_nofuse=True,
                            )
                        )
                    si.on_wait = waits[:limit]
                    inst.sync_info = si
                new_insts.append(inst)
            if changed:
                bb.instructions = new_insts


def _build_bass(apply_gamma_beta: bool, split_waits: bool = True):
    import concourse.bass as bass
    import concourse.mybir as mybir
    import concourse.tile as tile
    from concourse.bass import ts
    from concourse.vector_clock import ScopedClock

    PatchedTileContext = _patched_tile_context(tile, mybir, ScopedClock)
    f32 = mybir.dt.float32
    f32r = mybir.dt.float32r
    f16 = mybir.dt.float16

    fpm = f16 if PM_F16 else f32r
    nc = bass.Bass("TRN2")
    x = nc.dram_tensor("x", [T, D], f16, kind="ExternalInput")
    # w_proj.T / 7, centered over dout
    wT = nc.dram_tensor("wT", [D, D], f16, kind="ExternalInput")
    bias = nc.dram_tensor("bias", [1, D], f16, kind="ExternalInput")
    ones1 = nc.dram_tensor("ones1", [1, 128], f16, kind="ExternalInput")
    ident = nc.dram_tensor("ident", [128, 128], fpm, kind="ExternalInput")
    if apply_gamma_beta:
        gammaB = nc.dram_tensor("gammaB", [128, D], f32, kind="ExternalInput")
        betaB = nc.dram_tensor("betaB", [128, D], f32, kind="ExternalInput")
    out = nc.dram_tensor("out", [W, D], f16, kind="ExternalOutput")

    with PatchedTileContext(nc) as tc:
        with (
            tc.tile_pool(name="singles", bufs=1) as singles,
            tc.tile_pool(name="xr_pool", bufs=3) as xr_pool,
            tc.tile_pool(name="t_pool", bufs=2) as t_pool,
            tc.tile_pool(name="uv_pool", bufs=2) as uv_pool,
            tc.tile_pool(name="pm_pool", bufs=3) as pm_pool,
            tc.tile_pool(name="ptT_pool", bufs=2) as ptT_pool,
            tc.tile_pool(name="sq_pool", bufs=2) as sq_pool,
            tc.tile_pool(name="out_sb", bufs=3) as out_sb,
            tc.tile_pool(name="stat", bufs=6) as stat,
            tc.tile_pool(name="ps_t", bufs=3, space="PSUM") as ps_t,
            tc.tile_pool(name="ps_proj", bufs=3, space="PSUM") as ps_proj,
        ):
            # One-time loads on the ACT (scalar) HWDGE ring so the SP ring
            # starts streaming x immediately.
            id_sb = singles.tile([128, 128], fpm)
            nc.scalar.dma_start(out=id_sb[:], in_=ident[:, :])
            wt_sb = singles.tile([128, 4, D], f16)  # [d_lo, d_hi, dout]
            nc.scalar.dma_start(
                out=wt_sb[:], in_=wT[:, :].rearrange("(k p) n -> p k n", p=128)
            )
            bias_sb = singles.tile([1, D], f16)
            nc.scalar.dma_start(out=bias_sb[:], in_=bias[:, :])
            ones_sb = singles.tile([1, 128], f16)
            nc.scalar.dma_start(out=ones_sb[:], in_=ones1[:, :])
            eps_sb = singles.tile([128, 1], f32)
            nc.vector.memset(eps_sb[:], LN_EPS)
            # Warm-up: trigger the one-time ACT table load and PE state
            # load early, overlapped with the first x-chunk DMA.
            warm_sb = singles.tile([128, 1], f32)
            nc.scalar.activation(
                out=warm_sb[:],
                in_=eps_sb[:],
                func=mybir.ActivationFunctionType.Identity,
                scale=1.0,
            )
            warm_ps = ps_t.tile([128, 128], fpm, name="warm", bufs=1)
            nc.tensor.transpose(warm_ps[:], id_sb[:], id_sb[:])
            if apply_gamma_beta:
                g_sb = singles.tile([128, D], f32)
                nc.scalar.dma_start(out=g_sb[:], in_=gammaB[:, :])
                b_sb = singles.tile([128, D], f32)
                nc.scalar.dma_start(out=b_sb[:], in_=betaB[:, :])

            pms = {}

            def stage_a(c):
                # Double-chunk load: partition p holds all 8 rows of
                # segments 256c + p and 256c + 128 + p (two contiguous 8 KB
                # descriptors per partition).
                xr = xr_pool.tile([128, 2, STRIDE, D], f16, name="xr")
                xv = x[c * 2048 : (c + 1) * 2048, :].rearrange(
                    "(h s j) d -> s h j d", j=STRIDE, h=2
                )
                nc.sync.dma_start(out=xr[:], in_=xv[:, :, :, :])

                # Pooling over both chunks at once: fp16 add tree on DVE;
                # the space row (j=7) is excluded.
                with nc.allow_low_precision(reason="fp16 pooling tree"):
                    t = t_pool.tile([128, 2, 3, D], f16, name="t")
                    nc.vector.tensor_add(
                        t[:], xr[:, :, 0:5:2, :], xr[:, :, 1:6:2, :]
                    )
                    uv = uv_pool.tile([128, 2, 2, D], f16, name="uv")
                    nc.vector.tensor_add(
                        uv[:, :, 1, :], t[:, :, 2, :], xr[:, :, 6, :]
                    )
                    nc.vector.tensor_add(
                        uv[:, :, 0, :], t[:, :, 0, :], t[:, :, 1, :]
                    )
                    pm = pm_pool.tile([128, 2, D], fpm, name="pm")
                    nc.vector.tensor_add(pm[:], uv[:, :, 0, :], uv[:, :, 1, :])
                pms[2 * c] = (pm, 0)
                pms[2 * c + 1] = (pm, 1)

            pps = {}
            rstds = {}

            def stage_b1(R):
                pm, half = pms.pop(R)
                # pooled -> pooledT: 4 PE transposes into one PSUM bank,
                # one ACT copy to SBUF.
                ptp = ps_t.tile([128, 4, 128], fpm, name="ptp")
                for k in range(4):
                    nc.tensor.transpose(
                        ptp[:, k, :], pm[:, half, ts(k, 128)], id_sb[:]
                    )
                ptT = ptT_pool.tile([128, 4, 128], f16, name="ptT")
                nc.scalar.activation(
                    out=ptT[:],
                    in_=ptp[:],
                    func=mybir.ActivationFunctionType.Identity,
                    scale=1.0,
                )

                # projection for w-chunk R: psum[seg 128, dout 512]; the
                # bias lives in the accumulation group (K=1 ones x bias),
                # keeping the whole proj+bias on the PE (no extra hop).
                pp = ps_proj.tile([128, D], f32, name="pp")
                nc.tensor.matmul(
                    pp[:], lhsT=ones_sb[:], rhs=bias_sb[:], start=True, stop=False
                )
                for k in range(4):
                    nc.tensor.matmul(
                        pp[:],
                        lhsT=ptT[:, k, :],
                        rhs=wt_sb[:, k, :],
                        start=False,
                        stop=(k == 3),
                    )

                # Sum of squares via one ACT Square pass with row-accumulate,
                # then Sqrt on ACT; the tiny reciprocal runs on DVE one chunk
                # later (stage_b2) so the ACT->DVE->ACT zig is off the chain.
                sq = sq_pool.tile([128, D], f16, name="sq")
                ss = stat.tile([128, 1], f32, name="ss")
                nc.scalar.activation(
                    out=sq[:],
                    in_=pp[:],
                    func=mybir.ActivationFunctionType.Square,
                    accum_out=ss[:],
                )
                rstd = stat.tile([128, 1], f32, name="rstd")
                nc.scalar.activation(
                    out=rstd[:],
                    in_=ss[:],
                    func=mybir.ActivationFunctionType.Sqrt,
                    bias=eps_sb[:],
                    scale=1.0 / D,
                )
                pps[R] = pp
                rstds[R] = rstd

            def stage_b2(R):
                pp = pps.pop(R)
                rstd = rstds.pop(R)
                nc.vector.reciprocal(out=rstd[:], in_=rstd[:])
                if apply_gamma_beta:
                    ot32 = out_sb.tile([128, D], f32, name="ot32")
                    nc.scalar.activation(
                        out=ot32[:],
                        in_=pp[:],
                        func=mybir.ActivationFunctionType.Identity,
                        scale=rstd[:],
                    )
                    nc.vector.tensor_mul(out=ot32[:], in0=ot32[:], in1=g_sb[:])
                    ot = out_sb.tile([128, D], f16, name="ot")
                    nc.vector.tensor_add(out=ot[:], in0=ot32[:], in1=b_sb[:])
                else:
                    ot = out_sb.tile([128, D], f16, name="ot")
                    nc.scalar.activation(
                        out=ot[:],
                        in_=pp[:],
                        func=mybir.ActivationFunctionType.Identity,
                        scale=rstd[:],
                    )
                nc.scalar.dma_start(out=out[ts(R, 128), :], in_=ot[:])

            # Software-pipelined emission: loads/pooling (stage_a, per
            # double-chunk) run ~2 double-chunks ahead of the transpose/
            # proj/stats tail (stage_b1), which runs one chunk ahead of
            # the apply/store tail (stage_b2).
            schedule = [
                ("a", 0), ("a", 1),
                ("a", 2), ("b1", 0), ("b1", 1), ("b2", 0),
                ("a", 3), ("b1", 2), ("b2", 1), ("b1", 3), ("b2", 2),
                ("b1", 4), ("b2", 3), ("b1", 5), ("b2", 4),
                ("b1", 6), ("b2", 5), ("b1", 7), ("b2", 6), ("b2", 7),
            ]
            for kind, idx in schedule:
                if kind == "a":
                    stage_a(idx)
                elif kind == "b1":
                    stage_b1(idx)
                else:
                    stage_b2(idx)

    if split_waits:
        _split_multi_waits(nc, mybir)
    return nc


def _stage_inputs(inputs) -> tuple[bool, list[dict]]:
    """Host-side staging: fp16 x rows per core + replicated params."""
    x = np.asarray(inputs["x"], dtype=np.float32)
    w = np.asarray(inputs["w_proj"], dtype=np.float32)
    b = np.asarray(inputs["b_proj"], dtype=np.float32)
    gamma = np.asarray(inputs["gamma"], dtype=np.float32)
    beta = np.asarray(inputs["beta"], dtype=np.float32)
    assert x.shape == (B, T, D), x.shape

    apply_gb = not (np.all(gamma == 1.0) and np.all(beta == 0.0))
    # Center the projection over dout so the device matmul yields
    # proj - mean(proj) directly (mean is linear in the weights).
    wt = (w.T / VALID).astype(np.float64)
    wtc = wt - wt.mean(axis=1, keepdims=True)
    bc = b.astype(np.float64) - b.astype(np.float64).mean()
    common = {
        "wT": np.ascontiguousarray(wtc).astype(np.float16),
        "bias": np.ascontiguousarray(bc.reshape(1, D)).astype(np.float16),
        "ones1": np.ones((1, 128), dtype=np.float16),
        "ident": np.eye(128, dtype=np.float16 if PM_F16 else np.float32),
    }
    if apply_gb:
        common["gammaB"] = np.ascontiguousarray(
            np.broadcast_to(gamma.reshape(1, D), (128, D))
        )
        common["betaB"] = np.ascontiguousarray(
            np.broadcast_to(beta.reshape(1, D), (128, D))
        )
    x16 = x.astype(np.float16)
    in_maps = [
        {"x": np.ascontiguousarray(x16[i]), **common} for i in range(N_CORES)
    ]
    return apply_gb, in_maps


def kernel(**inputs) -> np.ndarray:
    from concourse.bass_utils import run_bass_kernel_spmd

    apply_gb, in_maps = _stage_inputs(inputs)
    nc = _build_bass(apply_gb)
    res = run_bass_kernel_spmd(nc, in_maps, core_ids=list(range(N_CORES)))
    return np.stack(
        [res.results[i]["out"].astype(np.float32) for i in range(N_CORES)], axis=0
    )


if __name__ == "__main__":
    rng = np.random.default_rng(0)
    demo = {
        "x": rng.standard_normal((B, T, D), dtype=np.float32),
        "input_ids": np.zeros((B, T), dtype=np.int64),
        "w_proj": rng.standard_normal((D, D), dtype=np.float32) / np.sqrt(D),
        "b_proj": (rng.standard_normal(D) * 0.01).astype(np.float32),
        "gamma": np.ones(D, dtype=np.float32),
        "beta": np.zeros(D, dtype=np.float32),
    }
    out = kernel(**demo)
    print(out.shape, out.dtype, float(np.abs(out).mean()))
